# revision 9
# baseline (speedup 1.0000x reference)
"""DeepseekV3-style MoE block on 8 Trainium2 NeuronCores.

Strategy (expert-parallel, host-side dispatch/combine):
  - Router (sigmoid + top-2 + normalize) computed on host in fp32. The
    routing decides the sharding itself: tokens are gathered per expert on
    the host (the "all-to-all dispatch") and each core e runs expert e's
    SwiGLU FFN over its gathered token batch (padded to a common capacity).
  - Shared expert: tensor-parallel 2-way over the hidden dim (HS=1024 ->
    two 512 halves) x data-parallel 4-way over tokens. Core e computes the
    ws-half (e // 4) over token slice (e % 4). The two halves of each token
    slice are summed on the host.
  - Combine: host scatter-adds routed outputs (scaled by routing weights)
    and adds shared partials.

Device kernel (identical program on all 8 cores): two SwiGLU FFN
evaluations [ntok,1024]x[1024,512] -> silu*mul -> [ntok,512]x[512,1024].
Everything is kept feature-major (features on SBUF partitions, tokens on
the free axis) so no transposes are needed:
    hT[h,t]  = sum_d W1[d,h] * xT[d,t]      (lhsT=W1 chunk, rhs=xT chunk)
    gT[h,t]  = silu(h1T) * h3T
    y[t,d]   = sum_h gT[h,t] * W2[h,d]      (lhsT=gT chunk,  rhs=W2 chunk)

All device inputs are host-packed into "chunk-panel" layout [128, n*cols]
(128-partition chunks laid side by side along the free dim) so each DMA
streams large contiguous per-partition rows at near-peak engine
throughput instead of one small packet per partition.

DMA routing (TRN2 has two HWDGE rings; per-core HBM read bandwidth
(~350 GB/s) is shared across rings, so splitting *inputs* across rings
just starves the stream that is needed first):
  - sync ring   : ALL inputs, in exact consumption order (w13 segments
    interleaved with xtr pairs, then w2, then v13 interleaved with xts,
    then v2).
  - scalar ring : all output tiles, so writes never queue behind reads.
A short PE warmup block (throwaway matmuls on a zeroed tile) runs while
the first inputs stream in: the HAM clock gate only un-throttles the PE
array (1.2 -> 2.4 GHz) after ~a full 4096-cycle window of dense
activity, so dense fake work up front buys the grant ~4us after kernel
start instead of ~15us into the real matmul stream.

PRECISION:
  - "bf16" (default): weights+activations+outputs bf16 on the wire; fp32
    PSUM accumulate. rel err ~4e-3.
  - "f32r": fp32 wire; matmuls in float32r. rel err ~2.6e-4.
  - "f32": exact fp32 matmuls (4 cyc/row). rel err ~1e-6.
"""

import os
import sys
from contextlib import ExitStack

import numpy as np

if "/opt/trn_rl_repo" not in sys.path and not os.path.isdir(
    os.path.join(os.path.dirname(os.path.abspath(__file__)), "concourse")
):
    sys.path.append("/opt/trn_rl_repo")

D = 1024  # model dim
E = 8  # experts
K = 2  # top-k
H = 512  # expert hidden
HS = 1024  # shared hidden
N_CORES = 8
TP_SHARED = 2  # shared expert split over HS
DP_SHARED = N_CORES // TP_SHARED  # shared expert split over tokens

PRECISION = os.environ.get("MOE_PRECISION", "bf16")

_NC_CACHE = {}
LAST_RUN = None  # BassKernelResults of the most recent kernel() call


def _pack_panels(a, chunk=128):
    """[n*chunk, cols] -> [chunk, n*cols] with row-chunk dc at column
    panel dc (panel[p, dc*cols + c] == a[dc*chunk + p, c])."""
    n = a.shape[0] // chunk
    return np.ascontiguousarray(
        a.reshape(n, chunk, a.shape[1]).transpose(1, 0, 2).reshape(chunk, -1)
    )


def _build_nc(cap, ts):
    """One-core Bass/Tile program: routed FFN over `cap` tokens + shared
    FFN half over `ts` tokens.

    DRAM inputs (host-packed chunk panels, feature-major):
      xtr  [128, 8*cap]   gathered routed tokens (D-chunk panels)
      xts  [128, 8*ts]    shared token slice     (D-chunk panels)
      w13  [128, 8*1024]  expert w1|w3           (D-chunk panels)
      w2p  [128, 4*1024]  expert w2              (H-chunk panels)
      v13  [128, 8*1024]  shared ws1|ws3 half    (D-chunk panels)
      v2p  [128, 4*1024]  shared ws2 half        (H-chunk panels)
    Outputs: yr [cap, D] (unscaled routed), ys [ts, D] (shared partial),
    both in the wire dtype.
    """
    import concourse.bacc as bacc
    import concourse.mybir as mybir
    import concourse.tile as tile

    f32 = mybir.dt.float32
    f32r = mybir.dt.float32r
    bf16 = mybir.dt.bfloat16
    AF = mybir.ActivationFunctionType

    wire = bf16 if PRECISION == "bf16" else f32

    def mm(ap):
        return ap.bitcast(f32r) if PRECISION == "f32r" else ap

    nc = bacc.Bacc("TRN2", target_bir_lowering=False)

    KC = D // 128  # contraction chunks for the first matmul
    HC = H // 128  # hidden chunks

    xtr = nc.declare_dram_parameter("xtr", [128, KC * cap], wire, isOutput=False)
    xts = nc.declare_dram_parameter("xts", [128, KC * ts], wire, isOutput=False)
    w13 = nc.declare_dram_parameter("w13", [128, KC * 2 * H], wire, isOutput=False)
    w2p = nc.declare_dram_parameter("w2p", [128, HC * D], wire, isOutput=False)
    v13 = nc.declare_dram_parameter("v13", [128, KC * 2 * H], wire, isOutput=False)
    v2p = nc.declare_dram_parameter("v2p", [128, HC * D], wire, isOutput=False)
    yr = nc.declare_dram_parameter("yr", [cap, D], wire, isOutput=True)
    ys = nc.declare_dram_parameter("ys", [ts, D], wire, isOutput=True)

    with ExitStack() as ctx:
        tc = ctx.enter_context(tile.TileContext(nc))
        wpool = ctx.enter_context(tc.tile_pool(name="w", bufs=1))
        xpool = ctx.enter_context(tc.tile_pool(name="x", bufs=1))
        gpool = ctx.enter_context(tc.tile_pool(name="g", bufs=2))
        spool = ctx.enter_context(tc.tile_pool(name="s", bufs=4))
        ypool = ctx.enter_context(tc.tile_pool(name="y", bufs=3))
        hps = ctx.enter_context(tc.tile_pool(name="hps", bufs=2, space="PSUM"))
        yps = ctx.enter_context(tc.tile_pool(name="yps", bufs=3, space="PSUM"))
        wps = ctx.enter_context(tc.tile_pool(name="wps", bufs=1, space="PSUM"))

        # Warm the PE's HAM clock gate while the first inputs stream in:
        # ~5us of dense throwaway matmuls on a zeroed tile get the 2.4 GHz
        # grant issued just as the first real chain becomes data-ready.
        warm = wpool.tile([128, 512], wire, tag="warm", name="warm")
        nc.vector.memset(warm[:], 0.0)
        wp = wps.tile([128, 512], f32, tag="wp", name="wp")
        for i in range(12):
            nc.tensor.matmul(
                wp[:],
                mm(warm[:, :128]),
                mm(warm[:]),
                start=(i == 0),
                stop=(i == 11),
            )

        def w_panel(width, pfx):
            return wpool.tile([128, width], wire, tag=pfx, name=pfx)

        def w_seg(eng, t, dram, width, nseg, i):
            seg = width // nseg
            eng.dma_start(
                mm(t[:, i * seg : (i + 1) * seg]),
                mm(dram[:, i * seg : (i + 1) * seg]),
            )

        def x_pair(eng, pool_dram, xw, pfx, j):
            """One [128, 2*xw] tile holding dc panels 2j and 2j+1."""
            t = xpool.tile([128, 2 * xw], wire, tag=f"{pfx}{j}", name=f"{pfx}{j}")
            eng.dma_start(mm(t[:]), mm(pool_dram[:, 2 * j * xw : (2 * j + 2) * xw]))
            return t

        # ---- Input DMA issue order == arrival order.  Both HWDGE rings
        # carry the stream in consumption order, segments alternating
        # between rings so each phase gets the full aggregate read BW. ----
        rings = [nc.sync, nc.scalar]

        w13_t = w_panel(KC * 2 * H, "w13")
        xtr_p = [None] * (KC // 2)
        for j in range(4):
            w_seg(rings[j % 2], w13_t, w13, KC * 2 * H, 4, j)
            xtr_p[j] = x_pair(rings[(j + 1) % 2], xtr, cap, "xtr", j)
        w2_t = w_panel(HC * D, "w2")
        for i in range(2):
            w_seg(rings[i % 2], w2_t, w2p, HC * D, 2, i)
        v13_t = w_panel(KC * 2 * H, "v13")
        xts_p = [None] * (KC // 2)
        for j in range(4):
            w_seg(rings[j % 2], v13_t, v13, KC * 2 * H, 4, j)
            xts_p[j] = x_pair(rings[(j + 1) % 2], xts, ts, "xts", j)
        v2_t = w_panel(HC * D, "v2")
        for i in range(2):
            w_seg(rings[i % 2], v2_t, v2p, HC * D, 2, i)

        def token_groups(ntok):
            """Split ntok (a multiple of 64) into free-dim groups <=512,
            keeping every group >=256 when possible so stage-A chains
            stay matmul-bound rather than LDWEIGHTS-bound."""
            n512, r = divmod(ntok, 512)
            gs = [512] * n512
            if r and r < 256 and n512 >= 1:
                gs[-1] = 256 + r
                r = 256
            if r:
                gs.append(r)
            return gs

        def stage_a(x_p, xw, wa_t, g0, F, pfx):
            """Emit the h1/h3 chains + silu*mul for one token group;
            returns the per-hc g tiles for stage_b."""

            def xsl(dc):
                return x_p[dc // 2][:, (dc % 2) * xw + g0 : (dc % 2) * xw + g0 + F]

            gs = []
            for hc in range(HC):
                h1 = hps.tile([128, 512], f32, tag="h1", name="h1")
                for dc in range(KC):
                    nc.tensor.matmul(
                        h1[:, :F],
                        mm(wa_t[:, dc * 1024 + hc * 128 : dc * 1024 + (hc + 1) * 128]),
                        mm(xsl(dc)),
                        start=(dc == 0),
                        stop=(dc == KC - 1),
                    )
                h3 = hps.tile([128, 512], f32, tag="h3", name="h3")
                for dc in range(KC):
                    nc.tensor.matmul(
                        h3[:, :F],
                        mm(wa_t[:, dc * 1024 + H + hc * 128 : dc * 1024 + H + (hc + 1) * 128]),
                        mm(xsl(dc)),
                        start=(dc == 0),
                        stop=(dc == KC - 1),
                    )
                s1 = spool.tile([128, 512], f32, tag="s1", name="s1")
                nc.scalar.activation(s1[:, :F], h1[:, :F], AF.Silu)
                g = gpool.tile([128, 512], wire, tag=f"g{hc}", name=f"g{pfx}{hc}")
                nc.vector.tensor_mul(mm(g[:, :F]), s1[:, :F], h3[:, :F])
                gs.append(g)
            return gs

        def stage_b(gs, w2a_t, out_dram, g0, F, oring):
            """Emit the down-projection + output DMA for one token group."""
            mt = 0
            while mt * 128 < F:
                tt = min(128, F - mt * 128)
                r0 = g0 + mt * 128
                y_sb = ypool.tile([128, D], wire, tag="ysb", name="ysb")
                for nh in range(2):
                    yp = yps.tile([128, 512], f32, tag="yp", name="yp")
                    for hc in range(HC):
                        nc.tensor.matmul(
                            yp[:tt],
                            mm(gs[hc][:, mt * 128 : mt * 128 + tt]),
                            mm(w2a_t[:, hc * 1024 + nh * 512 : hc * 1024 + (nh + 1) * 512]),
                            start=(hc == 0),
                            stop=(hc == HC - 1),
                        )
                    if nh == 0:
                        nc.scalar.activation(y_sb[:tt, 0:512], yp[:tt], AF.Copy)
                    else:
                        nc.vector.tensor_copy(y_sb[:tt, 512:1024], yp[:tt])
                rings[oring % 2].dma_start(out_dram[r0 : r0 + tt, :], y_sb[:tt, :])
                oring += 1
                mt += 1

        # Software-pipeline: stage B of group i is emitted after stage A of
        # group i+1, so its g tiles are long since ready (no silu->mul wait)
        # and stage A chains never stall behind stage-B PSUM pressure.
        jobs = []
        g0 = 0
        for F in token_groups(cap):
            jobs.append((xtr_p, cap, w13_t, w2_t, yr, g0, F, "r"))
            g0 += F
        g0 = 0
        for F in token_groups(ts):
            jobs.append((xts_p, ts, v13_t, v2_t, ys, g0, F, "s"))
            g0 += F

        pend = None
        oring = 0
        for x_p, xw, wa_t, w2a_t, out_dram, g0, F, pfx in jobs:
            gs = stage_a(x_p, xw, wa_t, g0, F, pfx)
            if pend is not None:
                stage_b(*pend)
                oring += (pend[4] + 127) // 128
            pend = (gs, w2a_t, out_dram, g0, F, oring)
        stage_b(*pend)

    nc.compile()
    return nc


def kernel(x, gate_w, w1, w3, w2, ws1, ws3, ws2):
    global LAST_RUN
    from concourse.bass_utils import run_bass_kernel_spmd

    x = np.asarray(x, dtype=np.float32)
    gate_w = np.asarray(gate_w, dtype=np.float32)
    w1 = np.asarray(w1, dtype=np.float32)
    w3 = np.asarray(w3, dtype=np.float32)
    w2 = np.asarray(w2, dtype=np.float32)
    ws1 = np.asarray(ws1, dtype=np.float32)
    ws3 = np.asarray(ws3, dtype=np.float32)
    ws2 = np.asarray(ws2, dtype=np.float32)

    if PRECISION == "bf16":
        import ml_dtypes

        wire_np = ml_dtypes.bfloat16
    else:
        wire_np = np.float32

    b, s, d = x.shape
    T = b * s
    xt = np.ascontiguousarray(x.reshape(T, d))
    ts = T // DP_SHARED  # shared-expert token slice per DP group

    # ---- Router on host (fp32, matches the jax reference's selection) ----
    logits = xt @ gate_w  # [T, E]
    with np.errstate(over="ignore"):
        scores = 1.0 / (1.0 + np.exp(-logits, dtype=np.float32))
    top2 = np.argpartition(-scores, 1, axis=1)[:, :2]  # top-2 set per token
    rows = np.arange(T)
    sel_scores = scores[rows[:, None], top2]  # [T, 2]
    norm_w = sel_scores / sel_scores.sum(axis=1, keepdims=True)

    tok_ids = []
    tok_w = []
    sel = np.zeros((T, E), dtype=bool)
    wmat = np.zeros((T, E), dtype=np.float32)
    sel[rows[:, None], top2] = True
    wmat[rows[:, None], top2] = norm_w
    for e in range(E):
        ids = np.nonzero(sel[:, e])[0]
        tok_ids.append(ids)
        tok_w.append(wmat[ids, e])

    max_ne = max(len(ids) for ids in tok_ids)
    cap = max(128, -(-max_ne // 64) * 64)

    # ---- Build per-core shards (chunk-panel packed, see _build_nc) ----
    xtT = np.ascontiguousarray(xt.T).astype(wire_np)  # [D, T]
    in_maps = []
    for e in range(E):
        ids = tok_ids[e]
        sl = e % DP_SHARED
        hf = e // DP_SHARED
        xe = np.zeros((d, cap), dtype=wire_np)
        xe[:, : len(ids)] = xtT[:, ids]
        w13 = np.concatenate([w1[e], w3[e]], axis=1).astype(wire_np)
        vs13 = np.concatenate(
            [ws1[:, hf * H : (hf + 1) * H], ws3[:, hf * H : (hf + 1) * H]],
            axis=1,
        ).astype(wire_np)
        in_maps.append(
            {
                "xtr": _pack_panels(xe),
                "xts": _pack_panels(
                    np.ascontiguousarray(xtT[:, sl * ts : (sl + 1) * ts])
                ),
                "w13": _pack_panels(w13),
                "w2p": _pack_panels(np.ascontiguousarray(w2[e]).astype(wire_np)),
                "v13": _pack_panels(vs13),
                "v2p": _pack_panels(
                    np.ascontiguousarray(ws2[hf * H : (hf + 1) * H, :]).astype(
                        wire_np
                    )
                ),
            }
        )

    key = (cap, ts, PRECISION)
    nc = _NC_CACHE.get(key)
    if nc is None:
        nc = _build_nc(cap, ts)
        _NC_CACHE[key] = nc

    last_err = None
    for _attempt in range(3):
        try:
            LAST_RUN = run_bass_kernel_spmd(nc, in_maps, list(range(N_CORES)))
            break
        except Exception as err:  # transient NRT/device failures: retry
            last_err = err
    else:
        raise last_err
    results = LAST_RUN.results

    # ---- Combine on host ----
    out = np.zeros((T, d), dtype=np.float32)
    for e in range(E):
        ids = tok_ids[e]
        yr_e = np.asarray(results[e]["yr"], dtype=np.float32)
        ys_e = np.asarray(results[e]["ys"], dtype=np.float32)
        out[ids] += yr_e[: len(ids)] * tok_w[e][:, None]
        sl = e % DP_SHARED
        out[sl * ts : (sl + 1) * ts] += ys_e
    return out.reshape(b, s, d)


# revision 13
# speedup vs baseline: 1.1021x; 1.1021x over previous
"""DeepseekV3-style MoE block on 8 Trainium2 NeuronCores.

Strategy (expert-parallel, host-side dispatch/combine):
  - Router (sigmoid + top-2 + normalize) computed on host in fp32. The
    routing decides the sharding itself: tokens are gathered per expert on
    the host (the "all-to-all dispatch") and each core e runs expert e's
    SwiGLU FFN over its gathered token batch (padded to a common capacity).
  - Shared expert: tensor-parallel 2-way over the hidden dim (HS=1024 ->
    two 512 halves) x data-parallel 4-way over tokens. Core e computes the
    ws-half (e // 4) over token slice (e % 4). The two halves of each token
    slice are summed on the host.
  - Combine: host scatter-adds routed outputs (scaled by routing weights)
    and adds shared partials.

Device kernel (identical program on all 8 cores): two SwiGLU FFN
evaluations [ntok,1024]x[1024,512] -> silu*mul -> [ntok,512]x[512,1024].
Everything is kept feature-major (features on SBUF partitions, tokens on
the free axis) so no transposes are needed:
    hT[h,t]  = sum_d W1[d,h] * xT[d,t]      (lhsT=W1 chunk, rhs=xT chunk)
    gT[h,t]  = silu(h1T) * h3T
    y[t,d]   = sum_h gT[h,t] * W2[h,d]      (lhsT=gT chunk,  rhs=W2 chunk)

All device inputs are host-packed into "chunk-panel" layout [128, n*cols]
(128-partition chunks laid side by side along the free dim) so each DMA
streams large contiguous per-partition rows at near-peak engine
throughput instead of one small packet per partition.

DMA routing (TRN2 has two HWDGE rings; per-core HBM read bandwidth
(~350 GB/s) is shared across rings, so splitting *inputs* across rings
just starves the stream that is needed first):
  - sync ring   : ALL inputs, in exact consumption order (w13 segments
    interleaved with xtr pairs, then w2, then v13 interleaved with xts,
    then v2).
  - scalar ring : all output tiles, so writes never queue behind reads.
A short PE warmup block (throwaway matmuls on a zeroed tile) runs while
the first inputs stream in: the HAM clock gate only un-throttles the PE
array (1.2 -> 2.4 GHz) after ~a full 4096-cycle window of dense
activity, so dense fake work up front buys the grant ~4us after kernel
start instead of ~15us into the real matmul stream.

PRECISION:
  - "bf16" (default): weights+activations+outputs bf16 on the wire; fp32
    PSUM accumulate. rel err ~4e-3.
  - "f32r": fp32 wire; matmuls in float32r. rel err ~2.6e-4.
  - "f32": exact fp32 matmuls (4 cyc/row). rel err ~1e-6.
"""

import os
import sys
from contextlib import ExitStack

import numpy as np

if "/opt/trn_rl_repo" not in sys.path and not os.path.isdir(
    os.path.join(os.path.dirname(os.path.abspath(__file__)), "concourse")
):
    sys.path.append("/opt/trn_rl_repo")

D = 1024  # model dim
E = 8  # experts
K = 2  # top-k
H = 512  # expert hidden
HS = 1024  # shared hidden
N_CORES = 8
TP_SHARED = 2  # shared expert split over HS
DP_SHARED = N_CORES // TP_SHARED  # shared expert split over tokens

PRECISION = os.environ.get("MOE_PRECISION", "bf16")

_NC_CACHE = {}
LAST_RUN = None  # BassKernelResults of the most recent kernel() call


def _pack_panels(a, chunk=128):
    """[n*chunk, cols] -> [chunk, n*cols] with row-chunk dc at column
    panel dc (panel[p, dc*cols + c] == a[dc*chunk + p, c])."""
    n = a.shape[0] // chunk
    return np.ascontiguousarray(
        a.reshape(n, chunk, a.shape[1]).transpose(1, 0, 2).reshape(chunk, -1)
    )


def _build_nc(cap, ts):
    """One-core Bass/Tile program: routed FFN over `cap` tokens + shared
    FFN half over `ts` tokens.

    DRAM inputs (host-packed chunk panels, feature-major):
      xtr  [128, 8*cap]   gathered routed tokens (D-chunk panels)
      xts  [128, 8*ts]    shared token slice     (D-chunk panels)
      w13  [128, 8*1024]  expert w1|w3           (D-chunk panels)
      w2p  [128, 4*1024]  expert w2              (H-chunk panels)
      v13  [128, 8*1024]  shared ws1|ws3 half    (D-chunk panels)
      v2p  [128, 4*1024]  shared ws2 half        (H-chunk panels)
    Outputs: yr [cap, D] (unscaled routed), ys [ts, D] (shared partial),
    both in the wire dtype.
    """
    import concourse.bacc as bacc
    import concourse.mybir as mybir
    import concourse.tile as tile

    f32 = mybir.dt.float32
    f32r = mybir.dt.float32r
    bf16 = mybir.dt.bfloat16
    AF = mybir.ActivationFunctionType

    wire = bf16 if PRECISION == "bf16" else f32

    def mm(ap):
        return ap.bitcast(f32r) if PRECISION == "f32r" else ap

    nc = bacc.Bacc("TRN2", target_bir_lowering=False)

    KC = D // 128  # contraction chunks for the first matmul
    HC = H // 128  # hidden chunks

    xtr = nc.declare_dram_parameter("xtr", [128, KC * cap], wire, isOutput=False)
    xts = nc.declare_dram_parameter("xts", [128, KC * ts], wire, isOutput=False)
    w13 = nc.declare_dram_parameter("w13", [128, KC * 2 * H], wire, isOutput=False)
    w2p = nc.declare_dram_parameter("w2p", [128, HC * D], wire, isOutput=False)
    v13 = nc.declare_dram_parameter("v13", [128, KC * 2 * H], wire, isOutput=False)
    v2p = nc.declare_dram_parameter("v2p", [128, HC * D], wire, isOutput=False)
    yr = nc.declare_dram_parameter("yr", [cap, D], wire, isOutput=True)
    ys = nc.declare_dram_parameter("ys", [ts, D], wire, isOutput=True)

    with ExitStack() as ctx:
        tc = ctx.enter_context(tile.TileContext(nc))
        wpool = ctx.enter_context(tc.tile_pool(name="w", bufs=1))
        xpool = ctx.enter_context(tc.tile_pool(name="x", bufs=1))
        gpool = ctx.enter_context(tc.tile_pool(name="g", bufs=2))
        spool = ctx.enter_context(tc.tile_pool(name="s", bufs=4))
        ypool = ctx.enter_context(tc.tile_pool(name="y", bufs=3))
        hps = ctx.enter_context(tc.tile_pool(name="hps", bufs=2, space="PSUM"))
        yps = ctx.enter_context(tc.tile_pool(name="yps", bufs=3, space="PSUM"))
        wps = ctx.enter_context(tc.tile_pool(name="wps", bufs=1, space="PSUM"))

        # Warm the PE's HAM clock gate while the first inputs stream in:
        # ~5us of dense throwaway matmuls on a zeroed tile get the 2.4 GHz
        # grant issued just as the first real chain becomes data-ready.
        warm = wpool.tile([128, 512], wire, tag="warm", name="warm")
        nc.vector.memset(warm[:], 0.0)
        wp = wps.tile([128, 512], f32, tag="wp", name="wp")
        for i in range(12):
            nc.tensor.matmul(
                wp[:],
                mm(warm[:, :128]),
                mm(warm[:]),
                start=(i == 0),
                stop=(i == 11),
            )

        def w_panel(width, pfx):
            return wpool.tile([128, width], wire, tag=pfx, name=pfx)

        def w_seg(eng, t, dram, width, nseg, i):
            seg = width // nseg
            eng.dma_start(
                mm(t[:, i * seg : (i + 1) * seg]),
                mm(dram[:, i * seg : (i + 1) * seg]),
            )

        def x_pair(eng, pool_dram, xw, pfx, j):
            """One [128, 2*xw] tile holding dc panels 2j and 2j+1."""
            t = xpool.tile([128, 2 * xw], wire, tag=f"{pfx}{j}", name=f"{pfx}{j}")
            eng.dma_start(mm(t[:]), mm(pool_dram[:, 2 * j * xw : (2 * j + 2) * xw]))
            return t

        # ---- Input DMA issue order == arrival order.  Both HWDGE rings
        # carry the stream in consumption order, segments alternating
        # between rings so each phase gets the full aggregate read BW. ----
        # All inputs on the sync ring: issuing input DMAs from the scalar
        # engine blocks its activation work behind DMA sem-lane-reuse
        # waits, which stalls the whole silu->mul->stage-B pipeline.
        w13_t = w_panel(KC * 2 * H, "w13")
        xtr_p = [None] * (KC // 2)
        for j in range(4):
            w_seg(nc.sync, w13_t, w13, KC * 2 * H, 4, j)
            xtr_p[j] = x_pair(nc.sync, xtr, cap, "xtr", j)
        w2_t = w_panel(HC * D, "w2")
        for i in range(2):
            w_seg(nc.sync, w2_t, w2p, HC * D, 2, i)
        v13_t = w_panel(KC * 2 * H, "v13")
        xts_p = [None] * (KC // 2)
        for j in range(4):
            w_seg(nc.sync, v13_t, v13, KC * 2 * H, 4, j)
            xts_p[j] = x_pair(nc.sync, xts, ts, "xts", j)
        v2_t = w_panel(HC * D, "v2")
        for i in range(2):
            w_seg(nc.sync, v2_t, v2p, HC * D, 2, i)

        def token_groups(ntok):
            """Split ntok (a multiple of 64) into free-dim groups <=512,
            keeping every group >=256 when possible so stage-A chains
            stay matmul-bound rather than LDWEIGHTS-bound."""
            n512, r = divmod(ntok, 512)
            gs = [512] * n512
            if r and r < 256 and n512 >= 1:
                gs[-1] = 256 + r
                r = 256
            if r:
                gs.append(r)
            return gs

        def stage_a(x_p, xw, wa_t, g0, F, pfx):
            """Emit the h1/h3 chains + silu*mul for one token group;
            returns the per-hc g tiles for stage_b."""

            def xsl(dc):
                return x_p[dc // 2][:, (dc % 2) * xw + g0 : (dc % 2) * xw + g0 + F]

            gs = []
            for hc in range(HC):
                h1 = hps.tile([128, 512], f32, tag="h1", name="h1")
                for dc in range(KC):
                    nc.tensor.matmul(
                        h1[:, :F],
                        mm(wa_t[:, dc * 1024 + hc * 128 : dc * 1024 + (hc + 1) * 128]),
                        mm(xsl(dc)),
                        start=(dc == 0),
                        stop=(dc == KC - 1),
                    )
                h3 = hps.tile([128, 512], f32, tag="h3", name="h3")
                for dc in range(KC):
                    nc.tensor.matmul(
                        h3[:, :F],
                        mm(wa_t[:, dc * 1024 + H + hc * 128 : dc * 1024 + H + (hc + 1) * 128]),
                        mm(xsl(dc)),
                        start=(dc == 0),
                        stop=(dc == KC - 1),
                    )
                s1 = spool.tile([128, 512], f32, tag="s1", name="s1")
                nc.scalar.activation(s1[:, :F], h1[:, :F], AF.Silu)
                g = gpool.tile([128, 512], wire, tag=f"g{hc}", name=f"g{pfx}{hc}")
                nc.vector.tensor_mul(mm(g[:, :F]), s1[:, :F], h3[:, :F])
                gs.append(g)
            return gs

        def stage_b(gs, w2a_t, out_dram, g0, F):
            """Emit the down-projection + output DMA for one token group."""
            mt = 0
            while mt * 128 < F:
                tt = min(128, F - mt * 128)
                r0 = g0 + mt * 128
                y_sb = ypool.tile([128, D], wire, tag="ysb", name="ysb")
                for nh in range(2):
                    yp = yps.tile([128, 512], f32, tag="yp", name="yp")
                    for hc in range(HC):
                        nc.tensor.matmul(
                            yp[:tt],
                            mm(gs[hc][:, mt * 128 : mt * 128 + tt]),
                            mm(w2a_t[:, hc * 1024 + nh * 512 : hc * 1024 + (nh + 1) * 512]),
                            start=(hc == 0),
                            stop=(hc == HC - 1),
                        )
                    if nh == 0:
                        nc.scalar.activation(y_sb[:tt, 0:512], yp[:tt], AF.Copy)
                    else:
                        nc.vector.tensor_copy(y_sb[:tt, 512:1024], yp[:tt])
                nc.scalar.dma_start(out_dram[r0 : r0 + tt, :], y_sb[:tt, :])
                mt += 1

        # Software-pipeline: stage B of group i is emitted after stage A of
        # group i+1, so its g tiles are long since ready (no silu->mul wait)
        # and stage A chains never stall behind stage-B PSUM pressure.
        jobs = []
        g0 = 0
        for F in token_groups(cap):
            jobs.append((xtr_p, cap, w13_t, w2_t, yr, g0, F, "r"))
            g0 += F
        g0 = 0
        for F in token_groups(ts):
            jobs.append((xts_p, ts, v13_t, v2_t, ys, g0, F, "s"))
            g0 += F

        pend = None
        for x_p, xw, wa_t, w2a_t, out_dram, g0, F, pfx in jobs:
            gs = stage_a(x_p, xw, wa_t, g0, F, pfx)
            if pend is not None:
                stage_b(*pend)
            pend = (gs, w2a_t, out_dram, g0, F)
        stage_b(*pend)

    nc.compile()
    return nc


def kernel(x, gate_w, w1, w3, w2, ws1, ws3, ws2):
    global LAST_RUN
    from concourse.bass_utils import run_bass_kernel_spmd

    x = np.asarray(x, dtype=np.float32)
    gate_w = np.asarray(gate_w, dtype=np.float32)
    w1 = np.asarray(w1, dtype=np.float32)
    w3 = np.asarray(w3, dtype=np.float32)
    w2 = np.asarray(w2, dtype=np.float32)
    ws1 = np.asarray(ws1, dtype=np.float32)
    ws3 = np.asarray(ws3, dtype=np.float32)
    ws2 = np.asarray(ws2, dtype=np.float32)

    if PRECISION == "bf16":
        import ml_dtypes

        wire_np = ml_dtypes.bfloat16
    else:
        wire_np = np.float32

    b, s, d = x.shape
    T = b * s
    xt = np.ascontiguousarray(x.reshape(T, d))
    ts = T // DP_SHARED  # shared-expert token slice per DP group

    # ---- Router on host (fp32, matches the jax reference's selection) ----
    logits = xt @ gate_w  # [T, E]
    with np.errstate(over="ignore"):
        scores = 1.0 / (1.0 + np.exp(-logits, dtype=np.float32))
    top2 = np.argpartition(-scores, 1, axis=1)[:, :2]  # top-2 set per token
    rows = np.arange(T)
    sel_scores = scores[rows[:, None], top2]  # [T, 2]
    norm_w = sel_scores / sel_scores.sum(axis=1, keepdims=True)

    tok_ids = []
    tok_w = []
    sel = np.zeros((T, E), dtype=bool)
    wmat = np.zeros((T, E), dtype=np.float32)
    sel[rows[:, None], top2] = True
    wmat[rows[:, None], top2] = norm_w
    for e in range(E):
        ids = np.nonzero(sel[:, e])[0]
        tok_ids.append(ids)
        tok_w.append(wmat[ids, e])

    max_ne = max(len(ids) for ids in tok_ids)
    cap = max(128, -(-max_ne // 64) * 64)

    # ---- Build per-core shards (chunk-panel packed, see _build_nc) ----
    xtT = np.ascontiguousarray(xt.T).astype(wire_np)  # [D, T]
    in_maps = []
    for e in range(E):
        ids = tok_ids[e]
        sl = e % DP_SHARED
        hf = e // DP_SHARED
        xe = np.zeros((d, cap), dtype=wire_np)
        xe[:, : len(ids)] = xtT[:, ids]
        w13 = np.concatenate([w1[e], w3[e]], axis=1).astype(wire_np)
        vs13 = np.concatenate(
            [ws1[:, hf * H : (hf + 1) * H], ws3[:, hf * H : (hf + 1) * H]],
            axis=1,
        ).astype(wire_np)
        in_maps.append(
            {
                "xtr": _pack_panels(xe),
                "xts": _pack_panels(
                    np.ascontiguousarray(xtT[:, sl * ts : (sl + 1) * ts])
                ),
                "w13": _pack_panels(w13),
                "w2p": _pack_panels(np.ascontiguousarray(w2[e]).astype(wire_np)),
                "v13": _pack_panels(vs13),
                "v2p": _pack_panels(
                    np.ascontiguousarray(ws2[hf * H : (hf + 1) * H, :]).astype(
                        wire_np
                    )
                ),
            }
        )

    key = (cap, ts, PRECISION)
    nc = _NC_CACHE.get(key)
    if nc is None:
        nc = _build_nc(cap, ts)
        _NC_CACHE[key] = nc

    last_err = None
    for _attempt in range(3):
        try:
            LAST_RUN = run_bass_kernel_spmd(nc, in_maps, list(range(N_CORES)))
            break
        except Exception as err:  # transient NRT/device failures: retry
            last_err = err
    else:
        raise last_err
    results = LAST_RUN.results

    # ---- Combine on host ----
    out = np.zeros((T, d), dtype=np.float32)
    for e in range(E):
        ids = tok_ids[e]
        yr_e = np.asarray(results[e]["yr"], dtype=np.float32)
        ys_e = np.asarray(results[e]["ys"], dtype=np.float32)
        out[ids] += yr_e[: len(ids)] * tok_w[e][:, None]
        sl = e % DP_SHARED
        out[sl * ts : (sl + 1) * ts] += ys_e
    return out.reshape(b, s, d)
